# revision 10
# baseline (speedup 1.0000x reference)
"""Bass/Trainium2 kernel for nn_BysMamba (bidirectional + stacked Mamba LM).

Sharding: tensor-parallel over d_inner ED=944 across 8 cores (118 channels
each). Every core keeps the full residual stream h (DIM=472 x B*L tokens,
fp32 master in DRAM), computes its channel shard of each Mamba block
(in_proj, causal conv as diagonal matmuls on the tensor engine, selective
scan via DVE tensor_tensor_scan with fp32 internal state, gating, out_proj
partial); partial x_proj / out_proj contractions are summed with AllReduce.
"""
import sys
sys.path.insert(0, '/opt/trn_rl_repo')

import numpy as np
import ml_dtypes

import concourse.bass as bass
from concourse import bacc
import concourse.mybir as mybir
import concourse.tile as tile
from concourse.masks import make_identity
from concourse.bass_utils import run_bass_kernel_spmd

F32 = mybir.dt.float32
BF16 = mybir.dt.bfloat16
AF = mybir.ActivationFunctionType
OP = mybir.AluOpType

V = 472
DIM = 472
ED = 944
NS = 16
KC = 4
R = 30
DEPTH = 8
B = 2

NCORES = 8
EC = ED // NCORES            # 118
MT = DIM // 4                # 118 residual row-tile
VOUT = DIM // NCORES         # 59 lm_head rows per core

SETS = ['in'] + [f'l{i}' for i in range(DEPTH)] + ['out']


def _bf(x):
    return np.ascontiguousarray(np.asarray(x, np.float32).astype(ml_dtypes.bfloat16))


def _f32(x):
    return np.ascontiguousarray(np.asarray(x, np.float32))


def prep_core_inputs(core, inputs, L):
    e0 = core * EC
    e1 = e0 + EC
    T = B * L
    d = {}
    x = np.asarray(inputs['x'], np.float32)           # (B, L, 3, 3)
    d['x_rhs'] = _bf(x.reshape(T, 9).T)               # (9, T)
    pw = np.asarray(inputs['patch_w'], np.float32)[:, 0].reshape(V, 9)
    d['patch_lhsT'] = _bf(pw.T)                       # (9, DIM)
    d['patch_b'] = _f32(np.asarray(inputs['patch_b']).reshape(4, MT, 1))
    lm = np.asarray(inputs['lm_head_w'], np.float32)[core * VOUT:(core + 1) * VOUT]
    d['lm_lhsT'] = _bf(lm.T.reshape(4, MT, VOUT))     # (4, MT, VOUT)
    for s in SETS:
        if s == 'in':
            g = lambda n: np.asarray(inputs[f'in_{n}'], np.float32)
        elif s == 'out':
            g = lambda n: np.asarray(inputs[f'out_{n}'], np.float32)
        else:
            li = int(s[1:])
            g = lambda n, li=li: np.asarray(inputs[f'lay_{n}'], np.float32)[li]
        ip = g('inproj_w')
        d[f'{s}_wxi'] = _bf(ip[e0:e1].T.reshape(4, MT, EC))
        d[f'{s}_wz'] = _bf(ip[ED + e0:ED + e1].T.reshape(4, MT, EC))
        cw = g('conv_w')[e0:e1, 0]                    # (EC, KC)
        diag = np.zeros((KC, EC, EC), np.float32)
        idx = np.arange(EC)
        for k in range(KC):
            diag[k, idx, idx] = cw[:, k]
        d[f'{s}_conv'] = _bf(diag)
        d[f'{s}_convb'] = _f32(g('conv_b')[e0:e1].reshape(EC, 1))
        d[f'{s}_xp'] = _bf(g('xproj_w')[:, e0:e1].T)  # (EC, 62)
        d[f'{s}_dt'] = _bf(g('dt_w')[e0:e1].T)        # (R, EC)
        d[f'{s}_dtb'] = _f32(g('dt_b')[e0:e1].reshape(EC, 1))
        d[f'{s}_A'] = _f32(-np.exp(g('Alog')[e0:e1])) # (EC, NS)
        d[f'{s}_D'] = _f32(g('D')[e0:e1].reshape(EC, 1))
        d[f'{s}_op'] = _bf(g('outproj_w')[:, e0:e1].T)  # (EC, DIM)
    return d


class Ctx:
    pass


def build_kernel(L):
    T = B * L
    nt = min(512, L)
    ntiles = T // nt
    jts = L // nt                # per-sample tiles

    nc = bacc.Bacc(num_devices=NCORES)
    din = {}

    def dram_in(name, shape, dt):
        din[name] = nc.dram_tensor(name, list(shape), dt, kind="ExternalInput")

    dram_in('x_rhs', (9, T), BF16)
    dram_in('patch_lhsT', (9, DIM), BF16)
    dram_in('patch_b', (4, MT, 1), F32)
    dram_in('lm_lhsT', (4, MT, VOUT), BF16)
    for s in SETS:
        dram_in(f'{s}_wxi', (4, MT, EC), BF16)
        dram_in(f'{s}_wz', (4, MT, EC), BF16)
        dram_in(f'{s}_conv', (KC, EC, EC), BF16)
        dram_in(f'{s}_convb', (EC, 1), F32)
        dram_in(f'{s}_xp', (EC, R + 2 * NS), BF16)
        dram_in(f'{s}_dt', (R, EC), BF16)
        dram_in(f'{s}_dtb', (EC, 1), F32)
        dram_in(f'{s}_A', (EC, NS), F32)
        dram_in(f'{s}_D', (EC, 1), F32)
        dram_in(f'{s}_op', (EC, DIM), BF16)
    out_t = nc.dram_tensor('out', [VOUT, T], F32, kind="ExternalOutput")

    c = Ctx()
    c.nc, c.din, c.out_t = nc, din, out_t
    c.L, c.T, c.nt, c.ntiles, c.jts = L, T, nt, ntiles, jts

    with tile.TileContext(nc) as tc:
        c.tc = tc
        with (
            tc.tile_pool(name="wp", bufs=1) as wp,
            tc.tile_pool(name="hp", bufs=2) as hp,
            tc.tile_pool(name="ap", bufs=1) as ap_,
            tc.tile_pool(name="sp", bufs=2) as sp,
            tc.tile_pool(name="pp", bufs=2, space="PSUM") as pp,
            tc.tile_pool(name="yp", bufs=1, space="PSUM") as yp,
            tc.tile_pool(name="dp", bufs=1, space="DRAM") as dp,
        ):
            c.wp, c.hp, c.ap, c.sp, c.pp, c.yp, c.dp = wp, hp, ap_, sp, pp, yp, dp

            W = {}
            for name, t in din.items():
                shp = list(t.shape)
                if len(shp) == 3:
                    wt = wp.tile([shp[1], shp[0], shp[2]], t.dtype, tag=f"w_{name}")
                    nc.sync.dma_start(wt[:], t[:].rearrange("k m e -> m k e"))
                else:
                    wt = wp.tile(shp, t.dtype, tag=f"w_{name}")
                    nc.sync.dma_start(wt[:], t[:])
                W[name] = wt
            c.W = W
            ident = wp.tile([EC, EC], BF16, tag="ident")
            make_identity(nc, ident[:])
            c.ident = ident

            c.h_dram = dp.tile([DIM, T], F32, tag="h_dram")
            c.cc_in = dp.tile([124, T], F32, tag="cc_in")
            c.cc_out = dp.tile([124, T], F32, tag="cc_out")
            c.up_in = dp.tile([DIM, T], BF16, tag="up_in")
            c.up_out = dp.tile([DIM, T], BF16, tag="up_out")

            build_body(c)
    nc.compile()
    return nc


def load_h_rhs(c, j):
    """Stage residual tokens [j*nt:(j+1)*nt) as bf16 rhs k-tiles [MT, 4, nt]."""
    nc = c.nc
    hbj = c.hp.tile([MT, 4, c.nt], BF16, tag="hbj")
    for k in range(4):
        nc.gpsimd.dma_start(hbj[:, k, :], c.h_dram[k * MT:(k + 1) * MT, bass.ts(j, c.nt)])
    return hbj


def build_body(c):
    nc = c.nc
    W = c.W
    nt = c.nt

    # ---- patch embedding ----
    xr = c.ap.tile([9, c.T], BF16, tag="xr")
    nc.sync.dma_start(xr[:], c.din['x_rhs'][:])
    for m in range(4):
        for j in range(c.ntiles):
            ps = c.pp.tile([MT, nt], F32, tag="ps")
            nc.tensor.matmul(ps[:], W['patch_lhsT'][:, bass.ts(m, MT)],
                             xr[:, bass.ts(j, nt)], start=True, stop=True)
            st = c.hp.tile([MT, nt], F32, tag="hstage")
            nc.scalar.activation(st[:], ps[:], AF.Identity, bias=W['patch_b'][:, m, :])
            nc.sync.dma_start(c.h_dram[m * MT:(m + 1) * MT, bass.ts(j, nt)], st[:])

    # ---- blocks ----
    run_pair(c, ['in', 'in'], [False, True])
    for i in range(DEPTH):
        run_pair(c, [f'l{i}'], [False])
    run_pair(c, ['out', 'out'], [False, True])

    # ---- lm head ----
    for j in range(c.ntiles):
        hbj = load_h_rhs(c, j)
        ps = c.pp.tile([VOUT, nt], F32, tag="ps")
        for k in range(4):
            nc.tensor.matmul(ps[:], W['lm_lhsT'][:, k, :], hbj[:, k, :],
                             start=(k == 0), stop=(k == 3))
        ot = c.hp.tile([VOUT, nt], F32, tag="lmout")
        nc.vector.tensor_copy(ot[:], ps[:])
        nc.sync.dma_start(c.out_t[:, bass.ts(j, nt)], ot[:])


def run_pair(c, sets, revs):
    """One mid layer (sets=[s]) or a bidir pair (sets=[s,s], revs=[F,T])."""
    nc = c.nc
    W = c.W
    L, nt, jts = c.L, c.nt, c.jts
    s0 = sets[0]
    pair = len(sets) == 2

    # ---- in_proj (shared between directions; flip commutes with pointwise) ----
    xi = [c.ap.tile([EC, L + 6], BF16, tag=f"xi{b}", name=f"xi{b}") for b in range(B)]
    for b in range(B):
        nc.gpsimd.memset(xi[b][:, 0:3], 0.0)
        nc.gpsimd.memset(xi[b][:, 3 + L:], 0.0)
    sz = c.ap.tile([EC, c.T], BF16, tag="sz")
    for j in range(c.ntiles):
        b, jj = divmod(j, jts)
        hbj = load_h_rhs(c, j)
        ps = c.pp.tile([EC, nt], F32, tag="ps")
        for k in range(4):
            nc.tensor.matmul(ps[:], W[f'{s0}_wxi'][:, k, :], hbj[:, k, :],
                             start=(k == 0), stop=(k == 3))
        nc.scalar.activation(xi[b][:, 3 + jj * nt:3 + (jj + 1) * nt], ps[:], AF.Copy)
        ps2 = c.pp.tile([EC, nt], F32, tag="ps")
        for k in range(4):
            nc.tensor.matmul(ps2[:], W[f'{s0}_wz'][:, k, :], hbj[:, k, :],
                             start=(k == 0), stop=(k == 3))
        nc.scalar.activation(sz[:, bass.ts(j, nt)], ps2[:], AF.Silu)

    # ---- per-direction conv + xproj partial ----
    xcs = []
    for di, (s, rev) in enumerate(zip(sets, revs)):
        xc = c.ap.tile([EC, c.T], BF16, tag=f"xc{di}")
        for j in range(c.ntiles):
            b, jj = divmod(j, jts)
            ps = c.pp.tile([EC, nt], F32, tag="ps")
            for k in range(KC):
                off = (6 - k) if rev else k
                nc.tensor.matmul(ps[:], W[f'{s}_conv'][:, k, :],
                                 xi[b][:, jj * nt + off: jj * nt + off + nt],
                                 start=(k == 0), stop=(k == KC - 1))
            nc.scalar.activation(xc[:, bass.ts(j, nt)], ps[:], AF.Silu,
                                 bias=W[f'{s}_convb'][:])
        xcs.append(xc)
        for j in range(c.ntiles):
            ps = c.pp.tile([62, nt], F32, tag="ps")
            nc.tensor.matmul(ps[:], W[f'{s}_xp'][:], xc[:, bass.ts(j, nt)],
                             start=True, stop=True)
            st = c.hp.tile([62, nt], F32, tag="dblst")
            nc.vector.tensor_copy(st[:], ps[:])
            nc.sync.dma_start(c.cc_in[62 * di:62 * (di + 1), bass.ts(j, nt)], st[:])

    # ---- merged AllReduce of x_proj partials ----
    rows = 124 if pair else 62
    nc.gpsimd.collective_compute(
        "AllReduce", OP.add, replica_groups=[list(range(NCORES))],
        ins=[c.cc_in[0:rows, :].opt()], outs=[c.cc_out[0:rows, :].opt()])

    # ---- per-direction: delta, scans, gating; accumulate y2sum ----
    y2sum = c.ap.tile([EC, c.T], BF16, tag="y2sum")
    for di, (s, rev) in enumerate(zip(sets, revs)):
        xc = xcs[di]
        dbl30 = c.ap.tile([R, c.T], BF16, tag="dbl30")
        nc.gpsimd.dma_start(dbl30[:], c.cc_out[62 * di:62 * di + R, :])

        delta = c.ap.tile([EC, c.T], BF16, tag="delta")
        spt = c.ap.tile([EC, c.T], BF16, tag="spt")
        for j in range(c.ntiles):
            ps = c.pp.tile([EC, nt], F32, tag="ps")
            nc.tensor.matmul(ps[:], W[f'{s}_dt'][:], dbl30[:, bass.ts(j, nt)],
                             start=True, stop=True)
            # softplus(x) = ln(1 + e^x); no Softplus table on this arch.
            # Exp over all tiles first, then Ln, to minimize table reloads.
            nc.scalar.activation(spt[:, bass.ts(j, nt)], ps[:], AF.Exp,
                                 bias=W[f'{s}_dtb'][:])
        for j in range(c.ntiles):
            nc.scalar.activation(delta[:, bass.ts(j, nt)], spt[:, bass.ts(j, nt)],
                                 AF.Ln, bias=1.0)
        u = c.ap.tile([EC, c.T], BF16, tag="u")
        nc.vector.tensor_mul(u[:], delta[:], xc[:])

        for b in range(B):
            ypss = [c.yp.tile([EC, nt], F32, tag=f"y{jj}", name=f"yps{jj}") for jj in range(jts)]
            for n in range(NS):
                ag = c.sp.tile([EC, L], BF16, tag="ag")
                bg = c.sp.tile([EC, L], BF16, tag="bg")
                hg = c.sp.tile([EC, L], BF16, tag="hg")
                brep = c.sp.tile([EC, L], BF16, tag="brep")
                crep = c.sp.tile([EC, L], BF16, tag="crep")
                nc.scalar.activation(ag[:], delta[:, b * L:(b + 1) * L], AF.Exp,
                                     scale=W[f'{s}_A'][:, n:n + 1])
                rb = 62 * di + R + n
                nc.gpsimd.dma_start(
                    brep[:, None, :],
                    c.cc_out[rb:rb + 1, b * L:(b + 1) * L].partition_broadcast(EC))
                nc.gpsimd.dma_start(
                    crep[:, None, :],
                    c.cc_out[rb + NS:rb + NS + 1, b * L:(b + 1) * L].partition_broadcast(EC))
                nc.vector.tensor_mul(bg[:], u[:, b * L:(b + 1) * L], brep[:])
                if rev:
                    nc.vector.tensor_tensor_scan(
                        hg[:, ::-1], ag[:, ::-1], bg[:, ::-1], 0.0, OP.mult, OP.add)
                else:
                    nc.vector.tensor_tensor_scan(
                        hg[:], ag[:], bg[:], 0.0, OP.mult, OP.add)
                nc.gpsimd.tensor_mul(hg[:], hg[:], crep[:])
                for jj in range(jts):
                    nc.tensor.matmul(ypss[jj][:], c.ident[:], hg[:, bass.ts(jj, nt)],
                                     start=(n == 0), stop=(n == NS - 1))
            for jj in range(jts):
                j = b * jts + jj
                y2p = c.hp.tile([EC, nt], BF16, tag="y2p")
                nc.vector.scalar_tensor_tensor(
                    y2p[:], xc[:, bass.ts(j, nt)], W[f'{s}_D'][:], ypss[jj][:],
                    op0=OP.mult, op1=OP.add)
                if di == 0:
                    nc.vector.tensor_mul(y2sum[:, bass.ts(j, nt)], y2p[:],
                                         sz[:, bass.ts(j, nt)])
                else:
                    nc.vector.tensor_mul(y2p[:], y2p[:], sz[:, bass.ts(j, nt)])
                    nc.vector.tensor_add(y2sum[:, bass.ts(j, nt)],
                                         y2sum[:, bass.ts(j, nt)], y2p[:])

    # ---- out_proj partial on y2sum ----
    for m in range(4):
        for j in range(c.ntiles):
            ps = c.pp.tile([MT, nt], F32, tag="ps")
            nc.tensor.matmul(ps[:], W[f'{s0}_op'][:, bass.ts(m, MT)],
                             y2sum[:, bass.ts(j, nt)], start=True, stop=True)
            st = c.hp.tile([MT, nt], BF16, tag="opst")
            nc.vector.tensor_copy(st[:], ps[:])
            nc.sync.dma_start(c.up_in[m * MT:(m + 1) * MT, bass.ts(j, nt)], st[:])

    nc.gpsimd.collective_compute(
        "AllReduce", OP.add, replica_groups=[list(range(NCORES))],
        ins=[c.up_in[:].opt()], outs=[c.up_out[:].opt()])

    # ---- residual update: h_dram += up_out ----
    for m in range(4):
        for j in range(c.ntiles):
            hs = c.hp.tile([MT, nt], F32, tag="hstage")
            nc.sync.dma_start(hs[:], c.h_dram[m * MT:(m + 1) * MT, bass.ts(j, nt)])
            us = c.hp.tile([MT, nt], BF16, tag="ustage")
            nc.sync.dma_start(us[:], c.up_out[m * MT:(m + 1) * MT, bass.ts(j, nt)])
            nc.vector.tensor_add(hs[:], hs[:], us[:])
            nc.sync.dma_start(c.h_dram[m * MT:(m + 1) * MT, bass.ts(j, nt)], hs[:])


_KERNEL_CACHE = {}


def get_kernel(L):
    if L not in _KERNEL_CACHE:
        _KERNEL_CACHE[L] = build_kernel(L)
    return _KERNEL_CACHE[L]


def kernel(**inputs):
    L = int(np.asarray(inputs['x']).shape[1])
    nc = get_kernel(L)
    in_maps = [prep_core_inputs(cc, inputs, L) for cc in range(NCORES)]
    res = run_bass_kernel_spmd(nc, in_maps, list(range(NCORES)))
    outs = [np.asarray(res.results[cc]['out'], np.float32) for cc in range(NCORES)]
    full = np.concatenate(outs, axis=0)                       # (V, T)
    return np.ascontiguousarray(full.reshape(V, B, L).transpose(1, 2, 0))


# revision 13
# speedup vs baseline: 245.4629x; 245.4629x over previous
"""Bass/Trainium2 kernel for nn_BysMamba (bidirectional + stacked Mamba LM).

Sharding: tensor-parallel over d_inner ED=944 across 8 cores (118 channels
each). Every core keeps the full residual stream h (DIM=472 x B*L tokens,
fp32 master in DRAM), computes its channel shard of each Mamba block
(in_proj, causal conv as diagonal matmuls on the tensor engine, selective
scan via DVE tensor_tensor_scan with fp32 internal state, gating, out_proj
partial); partial x_proj / out_proj contractions are summed with AllReduce.
"""
import sys
sys.path.insert(0, '/opt/trn_rl_repo')

import numpy as np
import ml_dtypes

import concourse.bass as bass
from concourse import bacc
import concourse.mybir as mybir
import concourse.tile as tile
from concourse.masks import make_identity
from concourse.bass_utils import run_bass_kernel_spmd

F32 = mybir.dt.float32
BF16 = mybir.dt.bfloat16
AF = mybir.ActivationFunctionType
OP = mybir.AluOpType

V = 472
DIM = 472
ED = 944
NS = 16
KC = 4
R = 30
DEPTH = 8
B = 2

NCORES = 8
EC = ED // NCORES            # 118
MT = DIM // 4                # 118 residual row-tile
VOUT = DIM // NCORES         # 59 lm_head rows per core

SETS = ['in'] + [f'l{i}' for i in range(DEPTH)] + ['out']


def _bf(x):
    return np.ascontiguousarray(np.asarray(x, np.float32).astype(ml_dtypes.bfloat16))


def _f32(x):
    return np.ascontiguousarray(np.asarray(x, np.float32))


def prep_core_inputs(core, inputs, L):
    e0 = core * EC
    e1 = e0 + EC
    T = B * L
    d = {}
    x = np.asarray(inputs['x'], np.float32)           # (B, L, 3, 3)
    d['x_rhs'] = _bf(x.reshape(T, 9).T)               # (9, T)
    pw = np.asarray(inputs['patch_w'], np.float32)[:, 0].reshape(V, 9)
    d['patch_lhsT'] = _bf(pw.T)                       # (9, DIM)
    d['patch_b'] = _f32(np.asarray(inputs['patch_b']).reshape(4, MT, 1))
    lm = np.asarray(inputs['lm_head_w'], np.float32)[core * VOUT:(core + 1) * VOUT]
    d['lm_lhsT'] = _bf(lm.T.reshape(4, MT, VOUT))     # (4, MT, VOUT)
    for s in SETS:
        if s == 'in':
            g = lambda n: np.asarray(inputs[f'in_{n}'], np.float32)
        elif s == 'out':
            g = lambda n: np.asarray(inputs[f'out_{n}'], np.float32)
        else:
            li = int(s[1:])
            g = lambda n, li=li: np.asarray(inputs[f'lay_{n}'], np.float32)[li]
        ip = g('inproj_w')
        d[f'{s}_wxi'] = _bf(ip[e0:e1].T.reshape(4, MT, EC))
        d[f'{s}_wz'] = _bf(ip[ED + e0:ED + e1].T.reshape(4, MT, EC))
        cw = g('conv_w')[e0:e1, 0]                    # (EC, KC)
        diag = np.zeros((KC, EC, EC), np.float32)
        idx = np.arange(EC)
        for k in range(KC):
            diag[k, idx, idx] = cw[:, k]
        d[f'{s}_conv'] = _bf(diag)
        d[f'{s}_convb'] = _f32(g('conv_b')[e0:e1].reshape(EC, 1))
        d[f'{s}_xp'] = _bf(g('xproj_w')[:, e0:e1].T)  # (EC, 62)
        d[f'{s}_dt'] = _bf(g('dt_w')[e0:e1].T)        # (R, EC)
        d[f'{s}_dtb'] = _f32(g('dt_b')[e0:e1].reshape(EC, 1))
        d[f'{s}_A'] = _f32(-np.exp(g('Alog')[e0:e1])) # (EC, NS)
        d[f'{s}_D'] = _f32(g('D')[e0:e1].reshape(EC, 1))
        d[f'{s}_op'] = _bf(g('outproj_w')[:, e0:e1].T)  # (EC, DIM)
    return d


class Ctx:
    pass


def build_kernel(L, repeat=1):
    T = B * L
    nt = min(512, L)
    ntiles = T // nt
    jts = L // nt                # per-sample tiles

    nc = bacc.Bacc(num_devices=NCORES)
    din = {}

    def dram_in(name, shape, dt):
        din[name] = nc.dram_tensor(name, list(shape), dt, kind="ExternalInput")

    dram_in('x_rhs', (9, T), BF16)
    dram_in('patch_lhsT', (9, DIM), BF16)
    dram_in('patch_b', (4, MT, 1), F32)
    dram_in('lm_lhsT', (4, MT, VOUT), BF16)
    for s in SETS:
        dram_in(f'{s}_wxi', (4, MT, EC), BF16)
        dram_in(f'{s}_wz', (4, MT, EC), BF16)
        dram_in(f'{s}_conv', (KC, EC, EC), BF16)
        dram_in(f'{s}_convb', (EC, 1), F32)
        dram_in(f'{s}_xp', (EC, R + 2 * NS), BF16)
        dram_in(f'{s}_dt', (R, EC), BF16)
        dram_in(f'{s}_dtb', (EC, 1), F32)
        dram_in(f'{s}_A', (EC, NS), F32)
        dram_in(f'{s}_D', (EC, 1), F32)
        dram_in(f'{s}_op', (EC, DIM), BF16)
    out_t = nc.dram_tensor('out', [VOUT, T], F32, kind="ExternalOutput")

    c = Ctx()
    c.nc, c.din, c.out_t = nc, din, out_t
    c.L, c.T, c.nt, c.ntiles, c.jts = L, T, nt, ntiles, jts

    with tile.TileContext(nc) as tc:
        c.tc = tc
        with (
            tc.tile_pool(name="wp", bufs=1) as wp,
            tc.tile_pool(name="hp", bufs=2) as hp,
            tc.tile_pool(name="ap", bufs=1) as ap_,
            tc.tile_pool(name="sp", bufs=2) as sp,
            tc.tile_pool(name="pp", bufs=2, space="PSUM") as pp,
            tc.tile_pool(name="yp", bufs=1, space="PSUM") as yp,
            tc.tile_pool(name="dp", bufs=1, space="DRAM") as dp,
        ):
            c.wp, c.hp, c.ap, c.sp, c.pp, c.yp, c.dp = wp, hp, ap_, sp, pp, yp, dp

            W = {}
            for name, t in din.items():
                shp = list(t.shape)
                if len(shp) == 3:
                    wt = wp.tile([shp[1], shp[0], shp[2]], t.dtype, tag=f"w_{name}")
                    nc.sync.dma_start(wt[:], t[:].rearrange("k m e -> m k e"))
                else:
                    wt = wp.tile(shp, t.dtype, tag=f"w_{name}")
                    nc.sync.dma_start(wt[:], t[:])
                W[name] = wt
            c.W = W
            ident = wp.tile([EC, EC], BF16, tag="ident")
            make_identity(nc, ident[:])
            c.ident = ident

            c.h_dram = dp.tile([DIM, T], F32, tag="h_dram")
            c.cc_in = dp.tile([124, T], BF16, tag="cc_in")
            c.cc_out = dp.tile([124, T], BF16, tag="cc_out")
            c.up_in = dp.tile([DIM, T], BF16, tag="up_in")
            c.up_out = dp.tile([DIM, T], BF16, tag="up_out")

            if repeat == 1:
                build_body(c)
            else:
                with tc.For_i(0, repeat, 1):
                    build_body(c)
    nc.compile()
    return nc


def load_h_rhs(c, j):
    """Stage residual tokens [j*nt:(j+1)*nt) as bf16 rhs k-tiles [MT, 4, nt]."""
    nc = c.nc
    hbj = c.hp.tile([MT, 4, c.nt], BF16, tag="hbj")
    nc.gpsimd.dma_start(
        hbj[:], c.h_dram[:, bass.ts(j, c.nt)].rearrange("(k m) t -> m k t", k=4))
    return hbj


def build_body(c):
    nc = c.nc
    W = c.W
    nt = c.nt

    # ---- patch embedding ----
    xr = c.ap.tile([9, c.T], BF16, tag="xr")
    nc.sync.dma_start(xr[:], c.din['x_rhs'][:])
    for m in range(4):
        for j in range(c.ntiles):
            ps = c.pp.tile([MT, nt], F32, tag="ps")
            nc.tensor.matmul(ps[:], W['patch_lhsT'][:, bass.ts(m, MT)],
                             xr[:, bass.ts(j, nt)], start=True, stop=True)
            st = c.hp.tile([MT, nt], F32, tag="hstage")
            nc.scalar.activation(st[:], ps[:], AF.Identity, bias=W['patch_b'][:, m, :])
            nc.sync.dma_start(c.h_dram[m * MT:(m + 1) * MT, bass.ts(j, nt)], st[:])

    # ---- blocks ----
    run_pair(c, ['in', 'in'], [False, True])
    for i in range(DEPTH):
        run_pair(c, [f'l{i}'], [False])
    run_pair(c, ['out', 'out'], [False, True])

    # ---- lm head ----
    for j in range(c.ntiles):
        hbj = load_h_rhs(c, j)
        ps = c.pp.tile([VOUT, nt], F32, tag="ps")
        for k in range(4):
            nc.tensor.matmul(ps[:], W['lm_lhsT'][:, k, :], hbj[:, k, :],
                             start=(k == 0), stop=(k == 3))
        ot = c.hp.tile([VOUT, nt], F32, tag="lmout")
        nc.vector.tensor_copy(ot[:], ps[:])
        nc.sync.dma_start(c.out_t[:, bass.ts(j, nt)], ot[:])


def run_pair(c, sets, revs):
    """One mid layer (sets=[s]) or a bidir pair (sets=[s,s], revs=[F,T])."""
    nc = c.nc
    W = c.W
    L, nt, jts = c.L, c.nt, c.jts
    s0 = sets[0]
    pair = len(sets) == 2

    # ---- in_proj (shared between directions; flip commutes with pointwise) ----
    xi = [c.ap.tile([EC, L + 6], BF16, tag=f"xi{b}", name=f"xi{b}") for b in range(B)]
    for b in range(B):
        nc.gpsimd.memset(xi[b][:, 0:3], 0.0)
        nc.gpsimd.memset(xi[b][:, 3 + L:], 0.0)
    sz = c.ap.tile([EC, c.T], BF16, tag="sz")
    for j in range(c.ntiles):
        b, jj = divmod(j, jts)
        hbj = load_h_rhs(c, j)
        ps = c.pp.tile([EC, nt], F32, tag="ps")
        for k in range(4):
            nc.tensor.matmul(ps[:], W[f'{s0}_wxi'][:, k, :], hbj[:, k, :],
                             start=(k == 0), stop=(k == 3))
        nc.scalar.activation(xi[b][:, 3 + jj * nt:3 + (jj + 1) * nt], ps[:], AF.Copy)
        ps2 = c.pp.tile([EC, nt], F32, tag="ps")
        for k in range(4):
            nc.tensor.matmul(ps2[:], W[f'{s0}_wz'][:, k, :], hbj[:, k, :],
                             start=(k == 0), stop=(k == 3))
        nc.scalar.activation(sz[:, bass.ts(j, nt)], ps2[:], AF.Silu)

    # ---- per-direction conv + xproj partial ----
    xcs = []
    for di, (s, rev) in enumerate(zip(sets, revs)):
        xc = c.ap.tile([EC, c.T], BF16, tag=f"xc{di}")
        for j in range(c.ntiles):
            b, jj = divmod(j, jts)
            ps = c.pp.tile([EC, nt], F32, tag="ps")
            for k in range(KC):
                off = (6 - k) if rev else k
                nc.tensor.matmul(ps[:], W[f'{s}_conv'][:, k, :],
                                 xi[b][:, jj * nt + off: jj * nt + off + nt],
                                 start=(k == 0), stop=(k == KC - 1))
            nc.scalar.activation(xc[:, bass.ts(j, nt)], ps[:], AF.Silu,
                                 bias=W[f'{s}_convb'][:])
        xcs.append(xc)
        for j in range(c.ntiles):
            ps = c.pp.tile([62, nt], F32, tag="ps")
            nc.tensor.matmul(ps[:], W[f'{s}_xp'][:], xc[:, bass.ts(j, nt)],
                             start=True, stop=True)
            st = c.hp.tile([62, nt], BF16, tag="dblst")
            nc.scalar.activation(st[:], ps[:], AF.Copy)
            nc.sync.dma_start(c.cc_in[62 * di:62 * (di + 1), bass.ts(j, nt)], st[:])

    # ---- merged AllReduce of x_proj partials ----
    rows = 124 if pair else 62
    nc.gpsimd.collective_compute(
        "AllReduce", OP.add, replica_groups=[list(range(NCORES))],
        ins=[c.cc_in[0:rows, :].opt()], outs=[c.cc_out[0:rows, :].opt()])

    # ---- per-direction: delta, scans, gating; accumulate y2sum ----
    y2sum = c.ap.tile([EC, c.T], BF16, tag="y2sum")
    for di, (s, rev) in enumerate(zip(sets, revs)):
        xc = xcs[di]
        dbl30 = c.ap.tile([R, c.T], BF16, tag="dbl30")
        nc.sync.dma_start(dbl30[:], c.cc_out[62 * di:62 * di + R, :])

        delta = c.ap.tile([EC, c.T], BF16, tag="delta")
        spt = c.ap.tile([EC, c.T], BF16, tag="spt")
        for j in range(c.ntiles):
            ps = c.pp.tile([EC, nt], F32, tag="ps")
            nc.tensor.matmul(ps[:], W[f'{s}_dt'][:], dbl30[:, bass.ts(j, nt)],
                             start=True, stop=True)
            # softplus(x) = ln(1 + e^x); no Softplus table on this arch.
            # Exp over all tiles first, then Ln, to minimize table reloads.
            nc.scalar.activation(spt[:, bass.ts(j, nt)], ps[:], AF.Exp,
                                 bias=W[f'{s}_dtb'][:])
        for j in range(c.ntiles):
            nc.scalar.activation(delta[:, bass.ts(j, nt)], spt[:, bass.ts(j, nt)],
                                 AF.Ln, bias=1.0)
        u = c.ap.tile([EC, c.T], BF16, tag="u")
        nc.vector.tensor_mul(u[:], delta[:], xc[:])

        for b in range(B):
            ypss = [c.yp.tile([EC, nt], F32, tag=f"y{jj}", name=f"yps{jj}") for jj in range(jts)]
            for n in range(NS):
                ag = c.sp.tile([EC, L], BF16, tag="ag")
                bg = c.sp.tile([EC, L], BF16, tag="bg")
                hg = c.sp.tile([EC, L], BF16, tag="hg")
                brep = c.sp.tile([EC, L], BF16, tag="brep")
                crep = c.sp.tile([EC, L], BF16, tag="crep")
                nc.scalar.activation(ag[:], delta[:, b * L:(b + 1) * L], AF.Exp,
                                     scale=W[f'{s}_A'][:, n:n + 1])
                rb = 62 * di + R + n
                nc.sync.dma_start(
                    brep[:, None, :],
                    c.cc_out[rb:rb + 1, b * L:(b + 1) * L].partition_broadcast(EC))
                nc.sync.dma_start(
                    crep[:, None, :],
                    c.cc_out[rb + NS:rb + NS + 1, b * L:(b + 1) * L].partition_broadcast(EC))
                nc.vector.tensor_mul(bg[:], u[:, b * L:(b + 1) * L], brep[:])
                if rev:
                    nc.vector.tensor_tensor_scan(
                        hg[:, ::-1], ag[:, ::-1], bg[:, ::-1], 0.0, OP.mult, OP.add)
                else:
                    nc.vector.tensor_tensor_scan(
                        hg[:], ag[:], bg[:], 0.0, OP.mult, OP.add)
                nc.gpsimd.tensor_mul(hg[:], hg[:], crep[:])
                for jj in range(jts):
                    nc.tensor.matmul(ypss[jj][:], c.ident[:], hg[:, bass.ts(jj, nt)],
                                     start=(n == 0), stop=(n == NS - 1))
            for jj in range(jts):
                j = b * jts + jj
                y2p = c.hp.tile([EC, nt], BF16, tag="y2p")
                nc.vector.scalar_tensor_tensor(
                    y2p[:], xc[:, bass.ts(j, nt)], W[f'{s}_D'][:], ypss[jj][:],
                    op0=OP.mult, op1=OP.add)
                if di == 0:
                    nc.vector.tensor_mul(y2sum[:, bass.ts(j, nt)], y2p[:],
                                         sz[:, bass.ts(j, nt)])
                else:
                    nc.vector.tensor_mul(y2p[:], y2p[:], sz[:, bass.ts(j, nt)])
                    nc.vector.tensor_add(y2sum[:, bass.ts(j, nt)],
                                         y2sum[:, bass.ts(j, nt)], y2p[:])

    # ---- out_proj partial on y2sum ----
    for m in range(4):
        for j in range(c.ntiles):
            ps = c.pp.tile([MT, nt], F32, tag="ps")
            nc.tensor.matmul(ps[:], W[f'{s0}_op'][:, bass.ts(m, MT)],
                             y2sum[:, bass.ts(j, nt)], start=True, stop=True)
            st = c.hp.tile([MT, nt], BF16, tag="opst")
            nc.scalar.activation(st[:], ps[:], AF.Copy)
            nc.sync.dma_start(c.up_in[m * MT:(m + 1) * MT, bass.ts(j, nt)], st[:])

    nc.gpsimd.collective_compute(
        "AllReduce", OP.add, replica_groups=[list(range(NCORES))],
        ins=[c.up_in[:].opt()], outs=[c.up_out[:].opt()])

    # ---- residual update: h_dram += up_out ----
    for m in range(4):
        for j in range(c.ntiles):
            hs = c.hp.tile([MT, nt], F32, tag="hstage")
            nc.sync.dma_start(hs[:], c.h_dram[m * MT:(m + 1) * MT, bass.ts(j, nt)])
            us = c.hp.tile([MT, nt], BF16, tag="ustage")
            nc.sync.dma_start(us[:], c.up_out[m * MT:(m + 1) * MT, bass.ts(j, nt)])
            nc.vector.tensor_add(hs[:], hs[:], us[:])
            nc.sync.dma_start(c.h_dram[m * MT:(m + 1) * MT, bass.ts(j, nt)], hs[:])


_KERNEL_CACHE = {}


def get_kernel(L, repeat=1):
    key = (L, repeat)
    if key not in _KERNEL_CACHE:
        _KERNEL_CACHE[key] = build_kernel(L, repeat)
    return _KERNEL_CACHE[key]


def kernel(**inputs):
    L = int(np.asarray(inputs['x']).shape[1])
    nc = get_kernel(L)
    in_maps = [prep_core_inputs(cc, inputs, L) for cc in range(NCORES)]
    res = run_bass_kernel_spmd(nc, in_maps, list(range(NCORES)))
    outs = [np.asarray(res.results[cc]['out'], np.float32) for cc in range(NCORES)]
    full = np.concatenate(outs, axis=0)                       # (V, T)
    return np.ascontiguousarray(full.reshape(V, B, L).transpose(1, 2, 0))
